# revision 20
# baseline (speedup 1.0000x reference)
"""BertSelfAttention Trainium2 kernel (8-core SPMD).

Problem: B=4, S=2048, HID=1024, H=16 heads, D=64.
Sharding: core c -> (batch b = c//2, head-group g = c%2). Each core computes
8 heads of one batch sample: QKV projections (512 out dims), scores^T,
softmax (denominator folded into the AV matmul via a ones column), AV.

Per-core layouts (all chosen so no on-device transposes are needed):
  xT   [1024, 2048]  = hidden[b].T               (bf16, host-prepped)
  wqT  [1024, 512]   = Wq[g*512:(g+1)*512].T     (bf16)
  qT/kT in SBUF as [128, 4, 2048]: partition = head-dim within head-pair
         (head 2*hp -> partitions 0-63, head 2*hp+1 -> partitions 64-127)
  v_aug in SBUF as [128, 16, 520]: per k-chunk, 8 heads x (64 dims + ones col)
  scores^T psum [128 k, 1024]: cols 0-511 head A, 512-1023 head B (same q)
  ctx^T  psum [65, 512] per (head, q-chunk): row 64 = softmax denominator
Output per core: [8, 65, 2048] fp32 (unnormalized ctx^T + sums row);
host divides by the sums row and transposes to [2048, 512].
"""

import numpy as np
import ml_dtypes

import concourse.bass as bass
import concourse.mybir as mybir
import concourse.tile as tile
from concourse import bacc, bass_utils

BF16 = mybir.dt.bfloat16
F32 = mybir.dt.float32

B, S, HID = 4, 2048, 1024
H, D = 16, 64
NCORES = 8
O = 512          # output dims per core (8 heads x 64)
HPC = 8          # heads per core
KC = HID // 128  # 8 contraction chunks for QKV
ST = S // 128    # 16 k-chunks in attention
QC = S // 512    # 4 q-chunks
OT = O // 128    # 4 head-pair tiles

_CACHE = {}


def _build():
    """Build the single-core Bass/Tile program (same NEFF on all 8 cores)."""
    from contextlib import ExitStack

    nc = bacc.Bacc("TRN2", target_bir_lowering=False, debug=False)

    xT_d = nc.dram_tensor("xT", [HID, S], BF16, kind="ExternalInput")
    wq_d = nc.dram_tensor("wqT", [HID, O], BF16, kind="ExternalInput")
    wk_d = nc.dram_tensor("wkT", [HID, O], BF16, kind="ExternalInput")
    wv_d = nc.dram_tensor("wvT", [HID, O], BF16, kind="ExternalInput")
    bq_d = nc.dram_tensor("bqc", [128, OT], F32, kind="ExternalInput")
    bk_d = nc.dram_tensor("bkc", [128, OT], F32, kind="ExternalInput")
    bv_d = nc.dram_tensor("bvb", [128, O], F32, kind="ExternalInput")
    mask_d = nc.dram_tensor("maskc", [128, ST], F32, kind="ExternalInput")
    out_d = nc.dram_tensor("ctxo", [HPC, D + 1, S], F32, kind="ExternalOutput")

    with tile.TileContext(nc) as tc, ExitStack() as ctx:
        sb = ctx.enter_context(tc.tile_pool(name="sb", bufs=1))
        epool = ctx.enter_context(tc.tile_pool(name="epool", bufs=7))
        opool = ctx.enter_context(tc.tile_pool(name="opool", bufs=4))
        qkv_ps = ctx.enter_context(tc.tile_pool(name="qkvps", bufs=2, space="PSUM"))
        s_ps = ctx.enter_context(tc.tile_pool(name="sps", bufs=2, space="PSUM"))
        ctx_ps = ctx.enter_context(tc.tile_pool(name="ctxps", bufs=2, space="PSUM"))

        from concourse.tile import add_dep_helper

        # ---- load everything to SBUF ----
        # DMA *issue* costs ~650 ns each on the SP sequencer and transfers get
        # ~110 GB/s per dma_start, so: issue the critical first-consumed data
        # (xtb0 + wv, interleaved per k-chunk) first and unchained; everything
        # else is chained behind xtb0's last chunk so it doesn't steal
        # bandwidth from the pipeline head (head-of-line blocking on the SP
        # queue is exactly the priority order we want).
        xsrc = xT_d.ap().rearrange("(kc p) s -> p kc s", p=128)
        wvsrc = wv_d.ap().rearrange("(kc p) n -> p kc n", p=128)
        xtb = [
            sb.tile([128, KC, 512], BF16, name=f"xtb{b}", tag=f"xtb{b}")
            for b in range(4)
        ]
        wv = sb.tile([128, KC, O], BF16, name="w_wv", tag="w_wv")
        x0_last = None
        for kc in range(0, KC, 2):
            nc.sync.dma_start(xtb[0][:, kc : kc + 2], xsrc[:, kc : kc + 2, 0:512])
            x0_last = nc.sync.dma_start(wv[:, kc : kc + 2], wvsrc[:, kc : kc + 2])

        bq_t = sb.tile([128, OT], F32, name="bq_t")
        nc.sync.dma_start(bq_t, bq_d.ap())
        bk_t = sb.tile([128, OT], F32, name="bk_t")
        nc.sync.dma_start(bk_t, bk_d.ap())
        bv_t = sb.tile([128, O], F32, name="bv_t")
        nc.sync.dma_start(bv_t, bv_d.ap())
        mask_t = sb.tile([128, ST], F32, name="mask_t")
        nc.sync.dma_start(mask_t, mask_d.ap())

        prev = x0_last
        later = []
        for b in range(1, 4):
            dma = nc.sync.dma_start(xtb[b], xsrc[:, :, b * 512 : (b + 1) * 512])
            add_dep_helper(dma.ins, prev.ins, sync=True,
                           reason="x block DMA priority chain")
            later.append(dma)
            prev = dma

        wts = []
        for i, (nm, d) in enumerate((("wq", wq_d), ("wk", wk_d))):
            w = sb.tile([128, KC, O], BF16, name=f"w_{nm}", tag=f"w_{nm}")
            wsrc = d.ap().rearrange("(kc p) n -> p kc n", p=128)
            for half in range(2):
                dma = nc.sync.dma_start(w[:, half * 4 : half * 4 + 4],
                                        wsrc[:, half * 4 : half * 4 + 4])
                add_dep_helper(dma.ins, later[i].ins, sync=True,
                               reason="wq/wk DMA after early x blocks")
            wts.append(w)
        wq, wk = wts

        qt = sb.tile([128, OT, S], BF16, name="qt")
        kt = sb.tile([128, OT, S], BF16, name="kt")
        vaug = sb.tile([128, ST, HPC * (D + 1)], BF16, name="vaug")

        # ---- V projection: v[s, o] = x @ Wv.T + bv, packed with ones col ----
        def emit_v_group(st):
            vps = qkv_ps.tile([128, 512], F32, name=f"vps{st}", tag="qkv")
            xb = xtb[st // 4]
            c0 = (st % 4) * 128
            for kc in range(KC):
                nc.tensor.matmul(
                    vps,
                    lhsT=xb[:, kc, c0 : c0 + 128],
                    rhs=wv[:, kc, :],
                    start=(kc == 0),
                    stop=(kc == KC - 1),
                )
            vv = vaug[:, st].rearrange("p (h c) -> p h c", c=D + 1)
            nc.vector.memset(vv[:, :, D], 1.0)
            nc.vector.tensor_add(
                out=vv[:, :, 0:D],
                in0=vps.rearrange("p (h d) -> p h d", d=D),
                in1=bv_t.rearrange("p (h d) -> p h d", d=D),
            )

        # ---- Q/K projections (transposed): qT[o, s] = WqT.T @ xT ----
        def emit_qk_chunks(hp, granularity=1):
            """Return list of small thunks (<=2 matmuls each) for head-pair hp."""
            thunks = []
            for proj in range(2):
                w = wq if proj == 0 else wk
                dest = qt if proj == 0 else kt
                bias = bq_t if proj == 0 else bk_t
                for sc in range(QC):
                    holder = {}

                    def mk_mm(k0, w=w, hp=hp, sc=sc, holder=holder, proj=proj):
                        def f():
                            if k0 == 0:
                                holder["ps"] = qkv_ps.tile(
                                    [128, 512], F32,
                                    name=f"qkps{proj}_{hp}_{sc}", tag="qkv",
                                )
                            ps = holder["ps"]
                            for kc in range(k0, k0 + granularity):
                                nc.tensor.matmul(
                                    ps,
                                    lhsT=w[:, kc, hp * 128 : (hp + 1) * 128],
                                    rhs=xtb[sc][:, kc, :],
                                    start=(kc == 0),
                                    stop=(kc == KC - 1),
                                    skip_group_check=True,
                                )
                        return f

                    for k0 in range(0, KC, granularity):
                        thunks.append(mk_mm(k0))

                    def mk_drain(dest=dest, bias=bias, hp=hp, sc=sc, holder=holder):
                        def f():
                            nc.vector.tensor_scalar(
                                out=dest[:, hp, sc * 512 : (sc + 1) * 512],
                                in0=holder["ps"],
                                scalar1=bias[:, hp : hp + 1],
                                scalar2=None,
                                op0=mybir.AluOpType.add,
                            )
                        return f

                    thunks.append(mk_drain())
            return thunks

        # ---- attention: one globally software-pipelined slot stream ----
        # Slot i carries scores+exp for flat index i (hp, qc, kc) and the AV
        # matmuls for index i-AVLAG, so the PE never waits on the ScalarE
        # exp and there are no pipeline bubbles at qc/hp boundaries.
        # Background QKV thunks (projections for the next head pair) are
        # paced to finish before their deadline and fill residual PE idle.
        AVLAG = 4
        NSLOT = OT * QC * ST
        etiles = {}
        ctxs = {}

        def scores_mm(idx):
            hp, r = divmod(idx, QC * ST)
            qc, kc = divmod(r, ST)
            s = s_ps.tile([128, 1024], F32, name=f"s{idx}", tag="s")
            nc.tensor.matmul(
                s[:, 0:512],
                lhsT=kt[0:64, hp, kc * 128 : (kc + 1) * 128],
                rhs=qt[0:64, hp, qc * 512 : (qc + 1) * 512],
                start=True, stop=True,
            )
            nc.tensor.matmul(
                s[:, 512:1024],
                lhsT=kt[64:128, hp, kc * 128 : (kc + 1) * 128],
                rhs=qt[64:128, hp, qc * 512 : (qc + 1) * 512],
                start=True, stop=True,
            )
            return s

        def exp_emit(idx, s):
            kc = idx % ST
            e = epool.tile([128, 1024], BF16, name=f"e{idx}", tag="e")
            nc.scalar.activation(
                e, s, mybir.ActivationFunctionType.Exp,
                bias=mask_t[:, kc : kc + 1],
                scale=float(1.0 / np.sqrt(D)),
            )
            etiles[idx] = e

        def av_emit(idx):
            hp, r = divmod(idx, QC * ST)
            qc, kc = divmod(r, ST)
            hA, hB = 2 * hp, 2 * hp + 1
            if kc == 0:
                ctxs[idx // ST] = (
                    ctx_ps.tile([D + 1, 512], F32, name=f"cA{hp}_{qc}", tag="ctx"),
                    ctx_ps.tile([D + 1, 512], F32, name=f"cB{hp}_{qc}", tag="ctx"),
                )
            cA, cB = ctxs[idx // ST]
            e = etiles.pop(idx)
            va = vaug[:, kc].rearrange("p (h c) -> p h c", c=D + 1)
            nc.tensor.matmul(
                cA, lhsT=va[:, hA], rhs=e[:, 0:512],
                start=(kc == 0), stop=(kc == ST - 1),
                skip_group_check=True,
            )
            nc.tensor.matmul(
                cB, lhsT=va[:, hB], rhs=e[:, 512:1024],
                start=(kc == 0), stop=(kc == ST - 1),
                skip_group_check=True,
            )
            if kc == ST - 1:
                del ctxs[idx // ST]
                for hh, cc in ((hA, cA), (hB, cB)):
                    stg = opool.tile([D + 1, 512], F32,
                                     name=f"stg{hh}_{qc}", tag="stg")
                    nc.vector.tensor_copy(out=stg, in_=cc)
                    nc.sync.dma_start(out_d[hh, :, qc * 512 : (qc + 1) * 512], stg)

        for st in range(ST):
            emit_v_group(st)
        for t in emit_qk_chunks(0):
            t()

        # bg thunks for head pair hp live in the slot window of hp-1 and must
        # finish ~4 slots before that window ends.
        bg_sched = []  # (deadline_pacer) per hp
        for hp in range(1, OT):
            thunks = emit_qk_chunks(hp)
            w0 = (hp - 1) * QC * ST
            span = QC * ST - 4
            bg_sched.append((w0, span, thunks, [0]))

        # Periods of 2 slots: the two slots' score pairs are emitted
        # back-to-back (4 adjacent row-tiled K=64 matmuls) so the PE can run
        # them concurrently in disjoint row groups, then the exps, background
        # thunks, and the previous period's AV matmuls.
        for p0 in range(0, NSLOT, 2):
            sA = scores_mm(p0)
            exp_emit(p0, sA)
            if p0 >= AVLAG:
                av_emit(p0 - AVLAG)
            sB = scores_mm(p0 + 1)
            exp_emit(p0 + 1, sB)
            for w0, span, thunks, done in bg_sched:
                if p0 < w0:
                    continue
                target = min(len(thunks), (p0 - w0 + 2) * len(thunks) // span + 1)
                while done[0] < target:
                    thunks[done[0]]()
                    done[0] += 1
            if p0 >= AVLAG:
                av_emit(p0 - AVLAG + 1)
        for idx in range(NSLOT - AVLAG, NSLOT):
            av_emit(idx)

    nc.compile()
    return nc


def _prep_core_inputs(hidden, mask, Wq, bq, Wk, bk, Wv, bv, b, g):
    bf16 = ml_dtypes.bfloat16
    o0 = g * O
    xT = np.ascontiguousarray(hidden[b].T).astype(bf16)
    ins = {
        "xT": xT,
        "wqT": np.ascontiguousarray(Wq[o0 : o0 + O].T).astype(bf16),
        "wkT": np.ascontiguousarray(Wk[o0 : o0 + O].T).astype(bf16),
        "wvT": np.ascontiguousarray(Wv[o0 : o0 + O].T).astype(bf16),
        "bqc": np.ascontiguousarray(
            bq[o0 : o0 + O].reshape(OT, 128).T).astype(np.float32),
        "bkc": np.ascontiguousarray(
            bk[o0 : o0 + O].reshape(OT, 128).T).astype(np.float32),
        "bvb": np.ascontiguousarray(
            np.broadcast_to(bv[o0 : o0 + O], (128, O))).astype(np.float32),
        "maskc": np.ascontiguousarray(
            mask[b, 0, 0, :].reshape(ST, 128).T).astype(np.float32),
    }
    return ins


def _postprocess(core_outs):
    """core_outs: list of 8 arrays [HPC, D+1, S] -> full [B, S, HID] fp32."""
    out = np.empty((B, S, HID), dtype=np.float32)
    for c in range(NCORES):
        b, g = c // 2, c % 2
        r = np.asarray(core_outs[c], dtype=np.float32)
        ctx = r[:, :D, :] / r[:, D : D + 1, :]      # normalize by softmax sums
        # [h, d, s] -> [s, h*64+d]
        out[b, :, g * O : (g + 1) * O] = ctx.transpose(2, 0, 1).reshape(S, O)
    return out


def get_nc():
    if "nc" not in _CACHE:
        _CACHE["nc"] = _build()
    return _CACHE["nc"]


def kernel(hidden_states, attention_mask, Wq, bq, Wk, bk, Wv, bv, **run_kwargs):
    hidden = np.asarray(hidden_states, dtype=np.float32)
    mask = np.asarray(attention_mask, dtype=np.float32)
    Wq = np.asarray(Wq, dtype=np.float32)
    Wk = np.asarray(Wk, dtype=np.float32)
    Wv = np.asarray(Wv, dtype=np.float32)
    bq = np.asarray(bq, dtype=np.float32)
    bk = np.asarray(bk, dtype=np.float32)
    bv = np.asarray(bv, dtype=np.float32)

    nc = get_nc()
    in_maps = [
        _prep_core_inputs(hidden, mask, Wq, bq, Wk, bk, Wv, bv, c // 2, c % 2)
        for c in range(NCORES)
    ]
    res = bass_utils.run_bass_kernel_spmd(
        nc, in_maps, core_ids=list(range(NCORES)), **run_kwargs
    )
    _CACHE["last_results"] = res
    return _postprocess([r["ctxo"] for r in res.results])
